# revision 1
# baseline (speedup 1.0000x reference)
"""Trainium2 Bass kernel for nn_ContinuousEmbedding (histogram binning + distance-
weighted embedding mix).

Math: for each scalar x[b,f], the reference computes bucket index
idx = #{j in 1..63 : x > low[j]} and returns
    out[b,f,:] = sum_k weight[k,:] / (|idx-k|+1)  =  T[idx,:]
where T = S @ weight, S[i,k] = 1/(|i-k|+1) is a fixed 64x64 matrix.

T[idx] telescopes over compare signs s_j = sign(x - low[j]) (s_0 = +1 since
low[0] = -inf):
    T[idx] = sum_j s_j * V2[j],  V2[0] = (T[0]+T[63])/2, V2[j] = (T[j]-T[j-1])/2
i.e. out_row = V2^T s(x) -- a 64-deep contraction the TensorEngine runs with V2
as a permanently-resident stationary and the sign grid streaming as the moving
operand. The device output is the transposed [D, tokens] layout; the host
transposes once at unshard time.

Per 1024-token chunk (64 chunks per core, processed in chunk pairs):
  grid:   xb[64, 1024] = x broadcast to 64 partitions, via either
            gpsimd.partition_broadcast (SBUF)  -- element-rate-bound ~1.6us
          or PE rank-1 outer product ones[1,64]^T @ xrow (PSUM) -- ~0.43us
          (mix is tuned so GPSIMD and PE finish together)
  sign:   sg[64, 1024] fp16 = Sign(xb + (-low))   (ACT, per-partition bias)
  gather: ps[128, 512] psum, col-tiled: chunk A -> partitions 0:64 via
          tile_position (0,0), chunk B -> 64:128 via (0,64); both V2 copies
          stay resident in separate PE column groups (no LDWEIGHTS churn).
  copy:   DVE psum -> sbuf [128, 1024]
  out:    2 HWDGE DMAs -> outT[64, NTOK] rows (4KB contiguous runs)
V2/-low are precomputed on the host from weight/low in float64. Tokens whose x
exactly equals a bin edge (sign(0)=0) are patched exactly on the host.
"""

import os as _os
import sys

import numpy as np

for _p in ("/opt/trn_rl_repo",):
    if _p not in sys.path:
        sys.path.insert(0, _p)

import concourse.bass as bass  # noqa: E402,F401
import concourse.mybir as mybir  # noqa: E402
import concourse.tile as tile  # noqa: E402
from concourse import bacc  # noqa: E402
from concourse import bass_utils  # noqa: E402

B, F, K, D = 8192, 64, 64, 64
NCORES = 8
NTOK = (B // NCORES) * F          # 65536 tokens per core
CHUNK = 1024                      # tokens per chunk
NPAIR = NTOK // (2 * CHUNK)       # 32 chunk pairs
HALF = CHUNK // 2                 # tokens per matmul (N=512)

FP16 = mybir.dt.float16
F32 = mybir.dt.float32

CFG = {
    "pe_pairs_mod8": 5,   # of every 8 chunk pairs, this many use the PE grid path
}
for _kv in _os.environ.get("KCFG", "").split(","):
    if "=" in _kv:
        _k, _v = _kv.split("=", 1)
        CFG[_k.strip()] = int(_v) if _v.strip().lstrip("-").isdigit() else _v.strip()


def build_tile_kernel(nc, tc, x_d, low_d, v_d, out_d):
    x_ap = x_d.ap().rearrange("(c n) -> c n", c=NTOK // CHUNK)       # [64, 1024]
    out_ap = out_d.ap().rearrange("d (c n) -> c d n", c=NTOK // CHUNK)

    with tc.tile_pool(name="cpool", bufs=1) as cpool:
        neglow = cpool.tile([K, 1], F32)
        nc.sync.dma_start(out=neglow[:], in_=low_d.ap())
        vtab = cpool.tile([K, D], FP16)
        nc.sync.dma_start(out=vtab[:], in_=v_d.ap())
        ones = cpool.tile([1, K], F32)
        nc.vector.memset(ones[:], 1.0)

        with (
            tc.tile_pool(name="wpool", bufs=3) as wpool,
            tc.tile_pool(name="spool", bufs=4) as spool,
            tc.tile_pool(name="opool", bufs=3) as opool,
            tc.tile_pool(name="pxpool", bufs=2, space="PSUM") as pxpool,
            tc.tile_pool(name="popool", bufs=2, space="PSUM") as popool,
        ):
            for p in range(NPAIR):
                pe_path = (p % 8) < CFG["pe_pairs_mod8"]
                sgs = []
                for half in range(2):
                    c = 2 * p + half
                    xrow = wpool.tile([1, CHUNK], F32, tag="xrow", bufs=4)
                    nc.sync.dma_start(out=xrow[:], in_=x_ap[c])
                    sg = spool.tile([K, CHUNK], FP16, tag=f"sg{half}")
                    if pe_path:
                        xbp = pxpool.tile([K, CHUNK], F32, tag="xbp")
                        for h in range(2):
                            nc.tensor.matmul(
                                out=xbp[:, HALF * h : HALF * (h + 1)],
                                lhsT=ones[:],
                                rhs=xrow[:, HALF * h : HALF * (h + 1)],
                                start=True,
                                stop=True,
                            )
                        src = xbp
                    else:
                        xb = wpool.tile([K, CHUNK], F32, tag="xb")
                        nc.gpsimd.partition_broadcast(xb[:], xrow[:], channels=K)
                        src = xb
                    nc.scalar.activation(
                        out=sg[:],
                        in_=src[:],
                        func=mybir.ActivationFunctionType.Sign,
                        bias=neglow[:],
                        scale=1.0,
                    )
                    sgs.append(sg)

                ps = popool.tile([128, CHUNK], F32, tag="ps")
                for half in range(2):
                    for h in range(2):
                        nc.tensor.matmul(
                            out=ps[64 * half : 64 * (half + 1), HALF * h : HALF * (h + 1)],
                            lhsT=vtab[:],
                            rhs=sgs[half][:, HALF * h : HALF * (h + 1)],
                            start=True,
                            stop=True,
                            tile_position=(0, 64 * half),
                        )

                ob = opool.tile([128, CHUNK], F32, tag="ob")
                nc.vector.tensor_copy(out=ob[:], in_=ps[:])
                for half in range(2):
                    nc.sync.dma_start(
                        out=out_ap[2 * p + half],
                        in_=ob[64 * half : 64 * (half + 1), :],
                    )


_CACHED_NC = None


def _get_nc():
    global _CACHED_NC
    if _CACHED_NC is None:
        nc = bacc.Bacc("TRN2", target_bir_lowering=False, debug=False)
        x_d = nc.dram_tensor("x", [NTOK], F32, kind="ExternalInput")
        low_d = nc.dram_tensor("lowcol", [K, 1], F32, kind="ExternalInput")
        v_d = nc.dram_tensor("vtab", [K, D], FP16, kind="ExternalInput")
        out_d = nc.dram_tensor("out", [D, NTOK], F32, kind="ExternalOutput")
        with tile.TileContext(nc) as tc:
            build_tile_kernel(nc, tc, x_d, low_d, v_d, out_d)
        nc.compile()
        _CACHED_NC = nc
    return _CACHED_NC


def make_host_tables(low, weight):
    """V2 [K, D] fp16 (sign-telescoped table) and -low column [K, 1] f32,
    computed in float64."""
    ar = np.arange(K)
    S = 1.0 / (np.abs(ar[:, None] - ar[None, :]) + 1.0)              # [K, K] f64
    T = S @ np.asarray(weight, np.float64)                           # [K, D]
    V = np.empty_like(T)
    V[0] = (T[0] + T[-1]) / 2
    V[1:] = (T[1:] - T[:-1]) / 2
    vtab = V.astype(np.float16)
    lowcol = (-np.asarray(low, np.float64)).astype(np.float32).reshape(K, 1)
    return lowcol, vtab


def host_correct_ties(out2d, xflat, low, weight):
    """Exact fixup for tokens where x equals a bin edge: the device Sign gives
    sign(0)=0 there (averaging two table rows) while the reference uses strict
    x > low. Replace those few rows with the exact table row."""
    bins = np.asarray(low, np.float32)[1:]
    ties = np.isin(xflat, bins)
    if not ties.any():
        return out2d
    xt = xflat[ties]
    idx = (xt[:, None] > bins[None, :]).sum(-1)
    ar = np.arange(K)
    S = 1.0 / (np.abs(ar[:, None] - ar[None, :]) + 1.0)
    T = (S @ np.asarray(weight, np.float64)).astype(np.float32)
    out2d[ties] = T[idx]
    return out2d


def run_cores(x, low, weight, trace=False):
    """Shard, run on 8 cores, return ([NTOK*8, D] f32 output, BassKernelResults)."""
    lowcol, vtab = make_host_tables(low, weight)
    nc = _get_nc()
    shards = np.asarray(x, np.float32).reshape(NCORES, NTOK)
    in_maps = [
        {"x": np.ascontiguousarray(shards[i]), "lowcol": lowcol, "vtab": vtab}
        for i in range(NCORES)
    ]
    res = bass_utils.run_bass_kernel_spmd(
        nc, in_maps, core_ids=list(range(NCORES)), trace=trace
    )
    out = np.concatenate(
        [np.ascontiguousarray(res.results[i]["out"].T) for i in range(NCORES)], axis=0
    )
    return out, res


def kernel(x, low, high, weight):
    x = np.asarray(x, np.float32)
    out, _ = run_cores(x, low, weight)
    out = host_correct_ties(out, x.reshape(-1), low, weight)
    return out.reshape(B, F, D)



# revision 3
# speedup vs baseline: 2.9504x; 2.9504x over previous
"""Trainium2 Bass kernel for nn_ContinuousEmbedding (histogram binning + distance-
weighted embedding mix).

Math: for each scalar x[b,f], the reference computes bucket index
idx = #{j in 1..63 : x > low[j]} and returns
    out[b,f,:] = sum_k weight[k,:] / (|idx-k|+1)  =  T[idx,:]
where T = S @ weight, S[i,k] = 1/(|i-k|+1) is a fixed 64x64 matrix.

T[idx] telescopes over compare signs s_j = sign(x - low[j]) (s_0 = +1 since
low[0] = -inf):
    T[idx] = sum_j s_j * V2[j],  V2[0] = (T[0]+T[63])/2, V2[j] = (T[j]-T[j-1])/2

Device pipeline (per superchunk = 2048 tokens: an A-chunk from the first half
of the core's tokens and a B-chunk from the second half, pair-packed into 128
partitions):
  bcast:  xb2[128, 1024] f32 psum = one bf16 matmul.  lhsT E_blk[6,128] is a
          0/1 selector; rhs rows are an exact 3-way bf16 split of x
          (hi+mid+lo == x exactly, bf16 shares f32's exponent range), so the
          f32 psum accumulation reconstructs x EXACTLY on 128 partitions
          (rows 0:64 = x_A, 64:128 = x_B).  No fp32 matmuls anywhere.
  sign:   alternating engines per superchunk (this is the throughput-critical
          pair of passes; ACT and DVE each do one pass per superchunk):
            ACT:  sg = Sign(xb2 + (-low))            in {-1, 0, +1}
            DVE:  sg = (xb2 + (-low)) >= 0           in {0, 1}
          Both sign-exact (f32 add is correctly rounded; only exact ties are
          wrong, patched on host).
  gather: one 128-deep block-diag fp16 matmul: lhsT = blockdiag(V2, V2) for
          the +/-1 grid or 2*blockdiag(V2,V2) for the {0,1} grid.
  copy:   the other engine copies psum -> fp16 sbuf; for {0,1} superchunks the
          -T[63] correction (V2^T 1 = T[63]) rides along as a per-partition
          bias/add.
  out:    coalesced 256KB fp16 DMAs (two superchunks per [64, 4KB-rows] DMA).
Host transposes [D, NTOK] -> [NTOK, D], casts fp16 -> f32, and patches exact
bin-edge ties.
"""

import os as _os
import sys

import numpy as np

for _p in ("/opt/trn_rl_repo",):
    if _p not in sys.path:
        sys.path.insert(0, _p)

import ml_dtypes  # noqa: E402

import concourse.bass as bass  # noqa: E402,F401
import concourse.mybir as mybir  # noqa: E402
import concourse.tile as tile  # noqa: E402
from concourse import bacc  # noqa: E402
from concourse import bass_utils  # noqa: E402

B, F, K, D = 8192, 64, 64, 64
NCORES = 8
NTOK = (B // NCORES) * F          # 65536 tokens per core
CHUNK = 1024                      # tokens per chunk
NSUP = NTOK // (2 * CHUNK)        # 32 superchunks (A-chunk + B-chunk each)
NGRP = NSUP // 2                  # 16 groups of 2 superchunks (one out-DMA pair)
HALF = 512                        # matmul free dim (one psum bank of f32)

BF16 = mybir.dt.bfloat16
FP16 = mybir.dt.float16
F32 = mybir.dt.float32
NPBF16 = ml_dtypes.bfloat16

CFG = {
    # sign engine per superchunk: ACT when (p % 2 == flip) else DVE
    "flip": 0,
    # every act_both_mod-th superchunk, ACT does BOTH sign+copy (shifts work
    # from DVE to the slightly faster ACT). 0 = off.
    "act_both_mod": 0,
}
for _kv in _os.environ.get("KCFG", "").split(","):
    if "=" in _kv:
        _k, _v = _kv.split("=", 1)
        CFG[_k.strip()] = int(_v) if _v.strip().lstrip("-").isdigit() else _v.strip()


def _sign_is_act(p):
    return (p % 2) == CFG["flip"]


def _copy_is_act(p):
    if CFG["act_both_mod"] and (p % CFG["act_both_mod"]) == (CFG["act_both_mod"] - 1):
        return True
    return not _sign_is_act(p)


def build_tile_kernel(nc, tc, xs_d, eblk_d, vblk_d, vgblk_d, neglow_d, negt63_d, out_d):
    xs_ap = xs_d.ap()                                             # [16, 6, 2048]
    out_ap = out_d.ap().rearrange("d (r n) -> r d n", r=2 * NGRP)  # [32, 64, 2048]

    with tc.tile_pool(name="cpool", bufs=1) as cpool:
        eblk = cpool.tile([6, 128], BF16)
        nc.sync.dma_start(out=eblk[:], in_=eblk_d.ap())
        vblk = cpool.tile([128, 128], FP16)
        nc.sync.dma_start(out=vblk[:], in_=vblk_d.ap())
        vgblk = cpool.tile([128, 128], FP16)
        nc.sync.dma_start(out=vgblk[:], in_=vgblk_d.ap())
        neglow = cpool.tile([128, 1], F32)
        nc.sync.dma_start(out=neglow[:], in_=neglow_d.ap())
        negt63 = cpool.tile([128, 1], F32)
        nc.sync.dma_start(out=negt63[:], in_=negt63_d.ap())

        with (
            tc.tile_pool(name="xpool", bufs=3) as xpool,
            tc.tile_pool(name="spool", bufs=3) as spool,
            tc.tile_pool(name="opool", bufs=3) as opool,
            tc.tile_pool(name="pxpool", bufs=2, space="PSUM") as pxpool,
            tc.tile_pool(name="popool", bufs=2, space="PSUM") as popool,
        ):
            xs_t = {}
            xb2 = {}
            sg = {}
            ot = {}

            def stage_front(p):
                """DMA-in (per group), bcast matmul, sign."""
                g, half = divmod(p, 2)
                if half == 0:
                    xs_t[g] = xpool.tile([6, 2048], BF16, tag="xs", name="xs_t")
                    nc.sync.dma_start(out=xs_t[g][:], in_=xs_ap[g])
                    ot[g] = opool.tile([128, 2048], FP16, tag="ot", name="ot")
                xb = pxpool.tile([128, CHUNK], F32, tag="xb")
                for h in range(2):
                    nc.tensor.matmul(
                        out=xb[:, HALF * h : HALF * (h + 1)],
                        lhsT=eblk[:],
                        rhs=xs_t[g][:, CHUNK * half + HALF * h : CHUNK * half + HALF * (h + 1)],
                        start=True,
                        stop=True,
                    )
                s = spool.tile([128, CHUNK], FP16, tag="sg")
                if _sign_is_act(p):
                    nc.scalar.activation(
                        out=s[:],
                        in_=xb[:],
                        func=mybir.ActivationFunctionType.Sign,
                        bias=neglow[:],
                        scale=1.0,
                    )
                else:
                    nc.vector.tensor_scalar(
                        out=s[:],
                        in0=xb[:],
                        scalar1=neglow[:],
                        scalar2=0.0,
                        op0=mybir.AluOpType.add,
                        op1=mybir.AluOpType.is_ge,
                    )
                xb2[p] = xb
                sg[p] = s

            def stage_back(p):
                """Gather matmul, psum->sbuf copy, out-DMA (per group)."""
                g, half = divmod(p, 2)
                act_grid = _sign_is_act(p)
                table = vblk if act_grid else vgblk
                ps = popool.tile([128, CHUNK], F32, tag="ps")
                for h in range(2):
                    nc.tensor.matmul(
                        out=ps[:, HALF * h : HALF * (h + 1)],
                        lhsT=table[:],
                        rhs=sg[p][:, HALF * h : HALF * (h + 1)],
                        start=True,
                        stop=True,
                    )
                dst = ot[g][:, CHUNK * half : CHUNK * (half + 1)]
                if _copy_is_act(p):
                    if act_grid:
                        nc.scalar.activation(
                            out=dst, in_=ps[:],
                            func=mybir.ActivationFunctionType.Copy,
                        )
                    else:
                        nc.scalar.activation(
                            out=dst, in_=ps[:],
                            func=mybir.ActivationFunctionType.Identity,
                            bias=negt63[:],
                            scale=1.0,
                        )
                else:
                    if act_grid:
                        nc.vector.tensor_copy(out=dst, in_=ps[:])
                    else:
                        nc.vector.tensor_scalar(
                            out=dst, in0=ps[:],
                            scalar1=negt63[:],
                            scalar2=None,
                            op0=mybir.AluOpType.add,
                        )
                del sg[p], xb2[p]
                if half == 1:
                    nc.sync.dma_start(out=out_ap[g], in_=ot[g][0:64, :])
                    nc.sync.dma_start(out=out_ap[NGRP + g], in_=ot[g][64:128, :])

            # software pipeline: front(p) runs one superchunk ahead of back(p)
            for p in range(NSUP + 1):
                if p < NSUP:
                    stage_front(p)
                if p >= 1:
                    stage_back(p - 1)


_CACHED_NC = None


def _get_nc():
    global _CACHED_NC
    if _CACHED_NC is None:
        nc = bacc.Bacc("TRN2", target_bir_lowering=False, debug=False)
        xs_d = nc.dram_tensor("xs", [NGRP, 6, 2048], BF16, kind="ExternalInput")
        eblk_d = nc.dram_tensor("eblk", [6, 128], BF16, kind="ExternalInput")
        vblk_d = nc.dram_tensor("vblk", [128, 128], FP16, kind="ExternalInput")
        vgblk_d = nc.dram_tensor("vgblk", [128, 128], FP16, kind="ExternalInput")
        neglow_d = nc.dram_tensor("neglow", [128, 1], F32, kind="ExternalInput")
        negt63_d = nc.dram_tensor("negt63", [128, 1], F32, kind="ExternalInput")
        out_d = nc.dram_tensor("out", [D, NTOK], FP16, kind="ExternalOutput")
        with tile.TileContext(nc) as tc:
            build_tile_kernel(
                nc, tc, xs_d, eblk_d, vblk_d, vgblk_d, neglow_d, negt63_d, out_d
            )
        nc.compile()
        _CACHED_NC = nc
    return _CACHED_NC


def make_host_tables(low, weight):
    """Constant device inputs, computed in float64."""
    ar = np.arange(K)
    S = 1.0 / (np.abs(ar[:, None] - ar[None, :]) + 1.0)              # [K, K] f64
    T = S @ np.asarray(weight, np.float64)                           # [K, D]
    V = np.empty_like(T)
    V[0] = (T[0] + T[-1]) / 2
    V[1:] = (T[1:] - T[:-1]) / 2

    vblk = np.zeros((128, 128), np.float64)
    vblk[0:64, 0:64] = V
    vblk[64:128, 64:128] = V
    vblk16 = vblk.astype(np.float16)
    vgblk16 = (2.0 * vblk).astype(np.float16)

    eblk = np.zeros((6, 128), np.float32)
    eblk[0:3, 0:64] = 1.0
    eblk[3:6, 64:128] = 1.0
    eblk16 = eblk.astype(NPBF16)

    lowfull = np.asarray(low, np.float64)                            # [-inf, bins]
    neg = np.where(np.isinf(lowfull), 3e38, -lowfull).astype(np.float32)
    neglow = np.concatenate([neg, neg]).reshape(128, 1)

    negt63 = np.concatenate([-T[63], -T[63]]).astype(np.float32).reshape(128, 1)
    return {
        "eblk": eblk16,
        "vblk": vblk16,
        "vgblk": vgblk16,
        "neglow": neglow,
        "negt63": negt63,
    }


def split_x_shard(shard):
    """Exact 3-way bf16 split of a [NTOK] f32 shard, arranged [NGRP, 6, 2048].

    Superchunk p pairs A-chunk p (tokens p*1024..) with B-chunk 32+p (tokens
    32768 + p*1024..).  Group g holds superchunks 2g (cols 0:1024) and 2g+1
    (cols 1024:2048); rows = hi/mid/lo of A then hi/mid/lo of B.
    """
    x = np.asarray(shard, np.float32)
    hi = x.astype(NPBF16).astype(np.float32)
    r = x - hi
    mid = r.astype(NPBF16).astype(np.float32)
    lo = (r - mid).astype(NPBF16)
    hi16 = hi.astype(NPBF16)
    mid16 = mid.astype(NPBF16)

    halfn = NTOK // 2
    parts = [hi16[:halfn], mid16[:halfn], lo[:halfn],
             hi16[halfn:], mid16[halfn:], lo[halfn:]]
    xs = np.empty((NGRP, 6, 2048), NPBF16)
    for r_i in range(6):
        # [32768] -> [16 groups, 2048 tokens] (natural order)
        xs[:, r_i, :] = parts[r_i].reshape(NGRP, 2048)
    return xs


def host_correct_ties(out2d, xflat, low, weight):
    """Exact fixup for tokens where x equals a bin edge: the device compare
    gives sign(0)=0 (ACT) or >=0 (DVE) there while the reference uses strict
    x > low. Replace those few rows with the exact table row."""
    bins = np.asarray(low, np.float32)[1:]
    ties = np.isin(xflat, bins)
    if not ties.any():
        return out2d
    xt = xflat[ties]
    idx = (xt[:, None] > bins[None, :]).sum(-1)
    ar = np.arange(K)
    S = 1.0 / (np.abs(ar[:, None] - ar[None, :]) + 1.0)
    T = (S @ np.asarray(weight, np.float64)).astype(np.float32)
    out2d[ties] = T[idx]
    return out2d


def build_in_maps(x, low, weight):
    consts = make_host_tables(low, weight)
    shards = np.asarray(x, np.float32).reshape(NCORES, NTOK)
    return [
        {"xs": split_x_shard(shards[i]), **consts}
        for i in range(NCORES)
    ]


def run_cores(x, low, weight, trace=False):
    """Shard, run on 8 cores, return ([NTOK*8, D] f32 output, BassKernelResults)."""
    nc = _get_nc()
    in_maps = build_in_maps(x, low, weight)
    res = bass_utils.run_bass_kernel_spmd(
        nc, in_maps, core_ids=list(range(NCORES)), trace=trace
    )
    out = np.concatenate(
        [
            np.ascontiguousarray(res.results[i]["out"].T).astype(np.float32)
            for i in range(NCORES)
        ],
        axis=0,
    )
    return out, res


def kernel(x, low, high, weight):
    x = np.asarray(x, np.float32)
    out, _ = run_cores(x, low, weight)
    out = host_correct_ties(out, x.reshape(-1), low, weight)
    return out.reshape(B, F, D)
